# revision 16
# baseline (speedup 1.0000x reference)
"""Trainium2 Bass kernel for nn_CrossAttention (B=4, Lq=Lk=2048, D=1024, H=16, d=64).

Sharding: 8 cores = 4 batches x 2 head-groups (8 heads each).
Each core computes a partial out^T = Wout_g^T @ y_g^T for its (batch, head-group);
host sums the two head-group partials per batch and transposes.

v2: single fused phase. The projection chunk pipeline (k+v and q sides
advancing in lockstep) interleaves attention (h, kc) units as soon as a
head-pair's qT/kT complete, so the PE's projection work fills the gaps of
the ACT-bound softmax stretch and the HAM clock-gate never re-throttles.
SQUARE runs on gpsimd from the bf16 raw copy, keeping ACT for exp (the
true floor). PSUM: one shared [128,1024] rotation tag (proj / v / var /
bcast / scores / D-bcast) + 4 pinned attnV accumulator banks = 8 banks.
Softmax denominators pack into a single [128,512] tile at row 32*qn+4*h,
reciprocated in one DVE op and broadcast in phase D via per-(fb,qn)
selector matmuls.
"""
import os
import numpy as np
from contextlib import ExitStack

import concourse.bass as bass
import concourse.tile as tile
from concourse import bacc, mybir
from concourse.bass_utils import run_bass_kernel_spmd

F32 = mybir.dt.float32
BF16 = mybir.dt.bfloat16
NP_BF16 = mybir.dt.np(BF16)
EXP = mybir.ActivationFunctionType.Exp
SQRT = mybir.ActivationFunctionType.Sqrt

D = 1024          # model dim
L = 2048          # seq len (q and k)
HC = 8            # heads per core
DH = 64           # head dim
F = HC * DH       # 512 local features
N_CORES = 8
EPS = float(np.finfo(np.float32).eps)

LAST_RESULTS = None  # BassKernelResults of the most recent run (for test harness)
_NC = None


# --------------------------------------------------------------------------- #
# Device program
# --------------------------------------------------------------------------- #

def _build_program(share_tables):
    nc = bacc.Bacc("TRN2", target_bir_lowering=False, debug=False,
                   num_devices=N_CORES)
    dt = nc.dram_tensor
    xqT = dt("xqT", (D, L), BF16, kind="ExternalInput").ap()
    xkvT = dt("xkvT", (D, L), BF16, kind="ExternalInput").ap()
    wq_d = dt("wq", (D, F), BF16, kind="ExternalInput").ap()
    wk_d = dt("wk", (D, F), BF16, kind="ExternalInput").ap()
    wv_d = dt("wv", (D, F), BF16, kind="ExternalInput").ap()
    wout = dt("wout", (F, D), BF16, kind="ExternalInput").ap()
    cq_d = dt("cq", (128, L), BF16, kind="ExternalInput").ap()
    sq_d = dt("sq", (128, L), BF16, kind="ExternalInput").ap()
    ck_d = dt("ck", (128, L), BF16, kind="ExternalInput").ap()
    sk_d = dt("sk", (128, L), BF16, kind="ExternalInput").ap()
    bdiag_d = dt("bdiag", (128, 128), BF16, kind="ExternalInput").ap()
    bmap_d = dt("bmap", (128, 128), BF16, kind="ExternalInput").ap()
    sel_d = [dt(f"sel{i}", (128, 128), BF16, kind="ExternalInput").ap()
             for i in range(2)]
    outT = dt("outT", (D, L), BF16, kind="ExternalOutput").ap()

    with tile.TileContext(nc) as tc:
        with ExitStack() as ctx:
            big = ctx.enter_context(tc.tile_pool(name="big", bufs=1))
            dram = ctx.enter_context(tc.tile_pool(name="dram", bufs=1,
                                                  space="DRAM"))
            pps = ctx.enter_context(
                tc.tile_pool(name="ps", bufs=1, space="PSUM"))
            attp = ctx.enter_context(tc.tile_pool(name="attp", bufs=1))

            # ---- persistent SBUF ----
            kT = [big.tile([128, L], BF16, tag=f"kT{i}", name=f"kT{i}")
                  for i in range(4)]
            qT = [big.tile([128, L], BF16, tag=f"qT{i}", name=f"qT{i}")
                  for i in range(4)]
            vaug = [big.tile([128, HC * 65], BF16, tag=f"v{i}",
                             name=f"vaug{i}") for i in range(16)]
            ytr = [big.tile([128, L], BF16, tag=f"ytr{i}", name=f"ytr{i}")
                   for i in range(4)]
            rk_dram = dram.tile([HC, L], F32, tag="rk")
            rk_sb = big.tile([128, HC, 16], F32, tag="rk_sb")
            # softmax denominators: per head-group tile, head h at
            # partition row 32*(h%4) (32-aligned rows: DVE partition
            # shifts are only legal at 32-row granularity)
            sums_g = [big.tile([128, L], BF16, tag=f"sums{g}",
                               name=f"sums{g}") for g in range(2)]
            bdiag = big.tile([128, 128], BF16, tag="bdiag")
            bmap = big.tile([128, 128], BF16, tag="bmap")
            sel_sb = [big.tile([128, 128], BF16, tag=f"sel{i}",
                               name=f"sel{i}") for i in range(2)]
            # manual double-buffered zero-padded rstd rows (rows 2-127 = 0)
            rstd_pad = [big.tile([128, 512], BF16, tag=f"rstdp{i}",
                                 name=f"rstdp{i}") for i in range(2)]
            eps_q = big.tile([2, 1], F32, tag="eps_q")
            eps_k = big.tile([2, 1], F32, tag="eps_k")

            nc.gpsimd.memset(sums_g[0][:], 1.0)
            nc.gpsimd.memset(sums_g[1][:], 1.0)
            nc.gpsimd.memset(rstd_pad[0][:], 0.0)
            nc.gpsimd.memset(rstd_pad[1][:], 0.0)
            nc.gpsimd.memset(eps_q[:], EPS)
            nc.gpsimd.memset(eps_k[:], 64.0 * EPS)

            nc.sync.dma_start(bdiag[:], bdiag_d[:])
            nc.sync.dma_start(bmap[:], bmap_d[:])
            for i in range(2):
                nc.sync.dma_start(sel_sb[i][:], sel_d[i][:])

            pctx = ExitStack()
            inp = pctx.enter_context(tc.tile_pool(name="inp", bufs=1))
            tmp = pctx.enter_context(tc.tile_pool(name="tmp", bufs=1))

            # ---- input DMAs: dc-interleaved, k-side first per dc ----
            wk_sb, xk_sb, wv_sb, wq_sb, xq_sb = [], [], [], [], []
            for dc in range(8):
                sl = slice(dc * 128, (dc + 1) * 128)
                for store, drt, nm, width in (
                        (wk_sb, wk_d, "wk", F), (xk_sb, xkvT, "xk", L),
                        (wv_sb, wv_d, "wv", F), (wq_sb, wq_d, "wq", F),
                        (xq_sb, xqT, "xq", L)):
                    t = inp.tile([128, width], BF16, tag=f"{nm}{dc}")
                    nc.sync.dma_start(t[:], drt[sl, :])
                    store.append(t)
            ck_sb = inp.tile([128, L], BF16, tag="ck")
            nc.sync.dma_start(ck_sb[:], ck_d[:])
            sk_sb = inp.tile([128, L], BF16, tag="sk")
            nc.sync.dma_start(sk_sb[:], sk_d[:])
            if share_tables:
                cq_sb, sq_sb = ck_sb, sk_sb
            else:
                cq_sb = inp.tile([128, L], BF16, tag="cq")
                nc.sync.dma_start(cq_sb[:], cq_d[:])
                sq_sb = inp.tile([128, L], BF16, tag="sq")
                nc.sync.dma_start(sq_sb[:], sq_d[:])

            # ---------------- projection stages ---------------- #
            kst = [dict() for _ in range(16)]
            qst = [dict() for _ in range(16)]

            def proj_stage1(c, side):
                fb, qc = c // 4, c % 4
                col0 = qc * 512
                w_sb, x_sb = (wk_sb, xk_sb) if side == "k" else (wq_sb, xq_sb)
                ps = pps.tile([128, 1024], F32, tag="ps", bufs=2)
                for dc in range(8):
                    nc.tensor.matmul(ps[:, 0:512],
                                     w_sb[dc][:, fb * 128:(fb + 1) * 128],
                                     x_sb[dc][:, col0:col0 + 512],
                                     start=(dc == 0), stop=(dc == 7))
                raw = tmp.tile([128, 512], BF16, tag=f"raw{side}", bufs=2)
                nc.vector.tensor_copy(raw[:], ps[:, 0:512])
                sq = tmp.tile([128, 512], BF16, tag=f"sq{side}", bufs=2)
                nc.gpsimd.tensor_mul(sq[:], raw[:], raw[:])
                st = kst[c] if side == "k" else qst[c]
                st.update(raw=raw, sq=sq, col0=col0, fb=fb)

            def stage1v(kc):
                ps = pps.tile([128, 1024], F32, tag="ps", bufs=2)
                for dc in range(8):
                    nc.tensor.matmul(ps[:, 0:512],
                                     xk_sb[dc][:, kc * 128:(kc + 1) * 128],
                                     wv_sb[dc][:],
                                     start=(dc == 0), stop=(dc == 7))
                va = vaug[kc]
                nc.gpsimd.memset(va[:], 1.0)
                va3 = va.rearrange("p (h c) -> p h c", c=65)
                ps3 = ps[:, 0:512].rearrange("p (h c) -> p h c", c=64)
                nc.vector.tensor_copy(va3[:, :, 0:64], ps3[:])

            def proj_stage2(c, side):
                st = kst[c] if side == "k" else qst[c]
                fb, col0 = st["fb"], st["col0"]
                c_sb, s_sb = (ck_sb, sk_sb) if side == "k" else (cq_sb, sq_sb)
                # rotate-half via 32-row-shifted DVE copies (sign folded
                # into the S table)
                raw = st["raw"]
                rot = tmp.tile([128, 512], BF16, tag=f"rot{side}", bufs=1)
                nc.vector.tensor_copy(rot[0:32, :], raw[32:64, :])
                nc.vector.tensor_copy(rot[32:64, :], raw[0:32, :])
                nc.vector.tensor_copy(rot[64:96, :], raw[96:128, :])
                nc.vector.tensor_copy(rot[96:128, :], raw[64:96, :])
                vps = pps.tile([128, 1024], F32, tag="ps", bufs=2)
                nc.tensor.matmul(vps[:, 0:512], bdiag[:], st["sq"][:],
                                 start=True, stop=True)
                std = tmp.tile([2, 512], F32, tag=f"std{side}", bufs=1)
                if side == "q":
                    nc.scalar.activation(std[:], vps[0:2, 0:512], SQRT,
                                         bias=eps_q[:], scale=1.0 / 64.0)
                else:
                    # fold the 1/8 attn scale: rk = 1/sqrt(64*(var/64+eps))
                    nc.scalar.activation(std[:], vps[0:2, 0:512], SQRT,
                                         bias=eps_k[:], scale=1.0)
                t2 = tmp.tile([128, 512], BF16, tag=f"t2{side}", bufs=1)
                nc.vector.tensor_mul(t2[:], rot[:],
                                     s_sb[:, col0:col0 + 512])
                t1 = tmp.tile([128, 512], BF16, tag=f"t1{side}", bufs=2)
                nc.gpsimd.tensor_mul(t1[:], st["raw"][:],
                                     c_sb[:, col0:col0 + 512])
                rstd = tmp.tile([2, 512], F32, tag=f"rstd{side}", bufs=1)
                nc.vector.reciprocal_approx_fast(out=rstd[:], in_=std[:])
                if side == "q":
                    rp = rstd_pad[c % 2]
                    nc.vector.tensor_copy(rp[0:2, :], rstd[:])
                    pre = tmp.tile([128, 512], BF16, tag="pre", bufs=2)
                    nc.gpsimd.tensor_add(pre[:], t1[:], t2[:])
                    st.update(pre=pre, rp=rp)
                else:
                    nc.gpsimd.dma_start(
                        rk_dram[2 * fb:2 * fb + 2, col0:col0 + 512], rstd[:])
                    nc.vector.tensor_add(kT[fb][:, col0:col0 + 512],
                                         t1[:], t2[:])
                    kst[c] = {}

            def q_stage3(c):
                st = qst[c]
                fb, col0 = st["fb"], st["col0"]
                bps = pps.tile([128, 1024], F32, tag="ps", bufs=2)
                nc.tensor.matmul(bps[:, 0:512], bmap[:], st["rp"][:],
                                 start=True, stop=True)
                nc.vector.tensor_mul(qT[fb][:, col0:col0 + 512],
                                     st["pre"][:], bps[:, 0:512])
                qst[c] = {}

            def rk_gather(fb):
                nc.sync.dma_start(
                    rk_sb[:, 2 * fb:2 * fb + 2, :],
                    rk_dram[2 * fb:2 * fb + 2, :].rearrange(
                        "h (kc p) -> p h kc", p=128))

            # ---------------- attention units ---------------- #
            va3s = [vaug[kc].rearrange("p (h c) -> p h c", c=65)
                    for kc in range(16)]
            att_state = {"h": None}

            def att_unit(h, kc):
                """scores+exp for (h, kc); attnV for (h, kc-1)."""
                fb, off = h // 2, (h % 2) * 64
                st = att_state
                if st["h"] != h:
                    st["h"] = h
                    st["yps"] = [pps.tile([128, 512], F32, tag=f"y{qn}",
                                          bufs=1, name=f"yps{h}_{qn}")
                                 [0:65, :] for qn in range(4)]
                    st["pend"] = None
                yps = st["yps"]
                if st["pend"] is not None:
                    pkc, ppts = st["pend"]
                    for half in range(2):
                        for j in range(2):
                            qn = half * 2 + j
                            nc.tensor.matmul(
                                yps[qn][:], va3s[pkc][:, h, :],
                                ppts[half][:, j * 512:(j + 1) * 512],
                                start=(pkc == 0), stop=False)
                pts = []
                for half in range(2):
                    sps = pps.tile([128, 1024], F32, tag="ps", bufs=2)
                    for j in range(2):
                        qn = half * 2 + j
                        nc.tensor.matmul(
                            sps[:, j * 512:(j + 1) * 512],
                            kT[fb][off:off + 64, kc * 128:(kc + 1) * 128],
                            qT[fb][off:off + 64, qn * 512:(qn + 1) * 512],
                            start=True, stop=True)
                    pt = attp.tile([128, 1024], BF16, tag="p", bufs=2)
                    nc.scalar.activation(pt[:], sps[:], EXP,
                                         scale=rk_sb[:, h, kc:kc + 1])
                    pts.append(pt)
                st["pend"] = (kc, pts)

            def att_finish(h):
                fb, off = h // 2, (h % 2) * 64
                st = att_state
                yps = st["yps"]
                pkc, ppts = st["pend"]
                for half in range(2):
                    for j in range(2):
                        qn = half * 2 + j
                        nc.tensor.matmul(
                            yps[qn][:], va3s[pkc][:, h, :],
                            ppts[half][:, j * 512:(j + 1) * 512],
                            start=False, stop=True)
                slot, g = 32 * (h % 4), h // 4
                for qn in range(4):
                    sl = slice(qn * 512, (qn + 1) * 512)
                    nc.vector.tensor_copy(ytr[fb][off:off + 64, sl],
                                          yps[qn][0:64, :])
                    nc.vector.tensor_copy(sums_g[g][slot:slot + 1, sl],
                                          yps[qn][64:65, :])
                    if h in (3, 7):
                        # group complete: replace sums with 1/sums in place
                        s32 = attp.tile([128, 512], F32, tag="s32", bufs=1)
                        nc.vector.tensor_copy(s32[:], sums_g[g][:, sl])
                        rs32 = attp.tile([128, 512], F32, tag="rs32", bufs=1)
                        nc.vector.reciprocal_approx_fast(
                            out=rs32[:], in_=s32[:])
                        nc.vector.tensor_copy(sums_g[g][:, sl], rs32[:])
                st["pend"] = None
                st["h"] = None

            # ---------------- fused driver ---------------- #
            units = []
            for h in range(HC):
                for kc in range(16):
                    units.append(("u", h, kc))
                units.append(("f", h, None))
            ucur = 0

            def unit_ready(u, s):
                kind, h, kc = u
                fb = h // 2
                if s < 4 * fb + 5:
                    return False
                if kind == "u" and kc > 2 * (s - 1):
                    return False
                return True

            def emit_units(s, budget):
                nonlocal ucur
                n = 0
                while n < budget and ucur < len(units) \
                        and unit_ready(units[ucur], s):
                    kind, h, kc = units[ucur]
                    if kind == "u":
                        att_unit(h, kc)
                    else:
                        att_finish(h)
                    ucur += 1
                    n += 1

            for s in range(16):
                proj_stage1(s, "k")
                if s < 8:
                    stage1v(2 * s)
                    stage1v(2 * s + 1)
                if s >= 1:
                    proj_stage2(s - 1, "k")
                proj_stage1(s, "q")
                if s >= 1:
                    proj_stage2(s - 1, "q")
                if s >= 2:
                    q_stage3(s - 2)
                if s % 4 == 0 and s >= 4:
                    rk_gather(s // 4 - 1)
                if s >= 6:
                    emit_units(s, 3)
            proj_stage2(15, "k")
            proj_stage2(15, "q")
            q_stage3(14)
            q_stage3(15)
            rk_gather(3)

            # remaining units for pairs 0-2 don't touch the input pool;
            # close it before loading wout so the load overlaps attention
            emit_units(99, max(0, len(units) - ucur - 40))
            pctx.close()

            dctx = ExitStack()
            cd = dctx.enter_context(tc.tile_pool(name="cd", bufs=1))
            wo_sb = []
            for fc in range(4):
                w = cd.tile([128, D], BF16, tag=f"wo{fc}")
                nc.sync.dma_start(w[:], wout[fc * 128:(fc + 1) * 128, :])
                wo_sb.append(w)
            emit_units(99, len(units) - ucur)

            # ---------------- phase D ---------------- #
            def d_stage1(qn):
                sl = slice(qn * 512, (qn + 1) * 512)
                for pair in range(2):
                    bt = pps.tile([128, 1024], F32, tag="ps", bufs=2,
                                  name=f"bc2_{pair}_{qn}")
                    for half in range(2):
                        fb = pair * 2 + half
                        bps = bt[:, half * 512:(half + 1) * 512]
                        nc.tensor.matmul(bps, sel_sb[fb % 2][:],
                                         sums_g[fb // 2][:, sl],
                                         start=True, stop=True)
                        nc.vector.tensor_mul(ytr[fb][:, sl],
                                             ytr[fb][:, sl], bps)

            def d_stage2(qn):
                sl = slice(qn * 512, (qn + 1) * 512)
                for nb in range(8):
                    ps = pps.tile([128, 512], F32, tag=f"y{nb % 4}",
                                  bufs=1, name=f"oproj_{nb}_{qn}")
                    for fc in range(4):
                        nc.tensor.matmul(
                            ps[:],
                            wo_sb[fc][:, nb * 128:(nb + 1) * 128],
                            ytr[fc][:, sl],
                            start=(fc == 0), stop=(fc == 3))
                    ot = cd.tile([128, 512], BF16, tag="ot", bufs=4)
                    nc.vector.tensor_copy(ot[:], ps[:])
                    eng = nc.sync if nb % 2 == 0 else nc.gpsimd
                    eng.dma_start(outT[nb * 128:(nb + 1) * 128, sl], ot[:])

            for i in range(5):
                if i < 4:
                    d_stage1(i)
                if i >= 1:
                    d_stage2(i - 1)
            dctx.close()
    nc.compile()
    return nc


def get_nc(share_tables=True):
    global _NC
    if _NC is None or _NC[1] != share_tables:
        _NC = (_build_program(share_tables), share_tables)
    return _NC[0]


# --------------------------------------------------------------------------- #
# Host side
# --------------------------------------------------------------------------- #

def _rope_tables(pos, g):
    """Feature-major folded RoPE(+gain) tables, replicated for a 2-head tile."""
    pos = np.asarray(pos).astype(np.float32)
    g = np.asarray(g, dtype=np.float32)
    inv = (1.0 / (10000.0 ** (np.arange(0, DH, 2, dtype=np.float32)
                              / np.float32(DH)))).astype(np.float32)
    ang = pos[:, None] * inv[None, :]                      # (L, 32)
    cos, sin = np.cos(ang, dtype=np.float32), np.sin(ang, dtype=np.float32)
    j = np.arange(DH)
    C = (g[j][:, None] * cos[:, j % 32].T).astype(np.float32)       # (64, L)
    sign = np.where(j < 32, -1.0, 1.0).astype(np.float32)
    S = (sign[:, None] * g[(j + 32) % 64][:, None]
         * sin[:, j % 32].T).astype(np.float32)
    return (np.ascontiguousarray(np.tile(C, (2, 1))).astype(NP_BF16),
            np.ascontiguousarray(np.tile(S, (2, 1))).astype(NP_BF16))


def make_in_maps(queries, kv, Wq, Wkv, Wout, g_q, g_k, pos_q, pos_k):
    queries = np.asarray(queries, dtype=np.float32)
    kv = np.asarray(kv, dtype=np.float32)
    Wq = np.asarray(Wq, dtype=np.float32)
    Wkv = np.asarray(Wkv, dtype=np.float32)
    Wout = np.asarray(Wout, dtype=np.float32)

    cq, sq = _rope_tables(pos_q, g_q)
    ck, sk = _rope_tables(pos_k, g_k)
    bdiag = np.zeros((128, 128), np.float32)
    bdiag[0:64, 0] = 1.0
    bdiag[64:128, 1] = 1.0
    bmap = np.zeros((128, 128), np.float32)
    bmap[0, 0:64] = 1.0
    bmap[1, 64:128] = 1.0
    # sums-row selectors: within its group tile, head h's denominators live
    # at row 32*(h%4); ytr[fb] rows 0:64 = head 2fb, 64:128 = head 2fb+1
    selA = np.zeros((128, 128), np.float32)
    selA[0, 0:64] = 1.0
    selA[32, 64:128] = 1.0
    selB = np.zeros((128, 128), np.float32)
    selB[64, 0:64] = 1.0
    selB[96, 64:128] = 1.0
    sels = [selA, selB]

    Wkv3 = Wkv.reshape(D, 16, 2 * DH)
    in_maps = []
    for c in range(N_CORES):
        b, grp = c // 2, c % 2
        hs = slice(grp * HC, (grp + 1) * HC)
        im = {
            "xqT": np.ascontiguousarray(queries[b].T).astype(NP_BF16),
            "xkvT": np.ascontiguousarray(kv[b].T).astype(NP_BF16),
            "wq": np.ascontiguousarray(
                Wq[:, grp * F:(grp + 1) * F]).astype(NP_BF16),
            "wk": np.ascontiguousarray(
                Wkv3[:, hs, :DH].reshape(D, F)).astype(NP_BF16),
            "wv": np.ascontiguousarray(
                Wkv3[:, hs, DH:].reshape(D, F)).astype(NP_BF16),
            "wout": np.ascontiguousarray(
                Wout[grp * F:(grp + 1) * F, :]).astype(NP_BF16),
            "cq": cq, "sq": sq, "ck": ck, "sk": sk,
            "bdiag": bdiag.astype(NP_BF16), "bmap": bmap.astype(NP_BF16),
        }
        for i in range(2):
            im[f"sel{i}"] = sels[i].astype(NP_BF16)
        in_maps.append(im)
    return in_maps


def kernel(queries, kv, Wq, Wkv, Wout, g_q, g_k, pos_q, pos_k):
    global LAST_RESULTS
    share = bool(np.array_equal(np.asarray(pos_q), np.asarray(pos_k))
                 and np.array_equal(np.asarray(g_q), np.asarray(g_k)))
    nc = get_nc(share)
    in_maps = make_in_maps(queries, kv, Wq, Wkv, Wout, g_q, g_k, pos_q, pos_k)
    trace = bool(int(os.environ.get("KERNEL_TRACE", "0")))
    kw = {}
    if trace:
        kw["tmpdir"] = os.environ.get("KERNEL_TRACE_DIR") or None
    res = run_bass_kernel_spmd(nc, in_maps, core_ids=list(range(N_CORES)),
                               trace=trace, **kw)
    LAST_RESULTS = res
    out = np.empty((4, L, D), np.float32)
    for b in range(4):
        out[b] = (res.results[2 * b]["outT"].astype(np.float32)
                  + res.results[2 * b + 1]["outT"].astype(np.float32)).T
    return out


# revision 22
# speedup vs baseline: 1.1122x; 1.1122x over previous
"""Trainium2 Bass kernel for nn_CrossAttention (B=4, Lq=Lk=2048, D=1024, H=16, d=64).

Sharding: 8 cores = 4 batches x 2 head-groups (8 heads each).
Each core computes a partial out^T = Wout_g^T @ y_g^T for its (batch, head-group);
host sums the two head-group partials per batch and transposes.

All matmuls run in bf16 (f32r costs ~2x bf16 on HW), the RoPE rotate-half is a
PE permutation matmul, and the projection phases are software-pipelined
(proj c | perm/var c-1 | bcast c-2) so the PE never waits on the
vector/scalar RMSNorm chain.  Both projection phases keep their input tiles
live simultaneously with all input DMAs issued up front — otherwise phase B's
loads alias phase A's SBUF and stall until A's last consumer retires (21us PE
gap + a HAM re-throttle).  PSUM rotates through one shared 8-bank pool.

Device layout is feature-major ("T" = [feature, seq]) throughout:
  qT/kT: [512, L] bf16 (8 heads x 64 dims on partitions, seq on free axis)
  S^T:   [k, q] tiles -> softmax sum via an appended ones-column in v (M=65)
  exp:   ACT, with the k-side RMSNorm rstd (and the 1/sqrt(d) scale) folded
         into the per-partition activation scale operand.
"""
import os
import numpy as np
from contextlib import ExitStack

import concourse.bass as bass
import concourse.tile as tile
from concourse import bacc, mybir
from concourse.bass_utils import run_bass_kernel_spmd

F32 = mybir.dt.float32
BF16 = mybir.dt.bfloat16
NP_BF16 = mybir.dt.np(BF16)
EXP = mybir.ActivationFunctionType.Exp
SQUARE = mybir.ActivationFunctionType.Square
SQRT = mybir.ActivationFunctionType.Sqrt
COPYF = mybir.ActivationFunctionType.Copy

D = 1024          # model dim
L = 2048          # seq len (q and k)
HC = 8            # heads per core
DH = 64           # head dim
F = HC * DH       # 512 local features
N_CORES = 8
EPS = float(np.finfo(np.float32).eps)

LAST_RESULTS = None  # BassKernelResults of the most recent run (for test harness)
_NC = None


# --------------------------------------------------------------------------- #
# Device program
# --------------------------------------------------------------------------- #

def _proj_dmas(nc, inp, x_dram, w_dram, c_dram, s_dram, side, wv_dram=None,
               tables=None):
    """Issue the input DMAs for one projection side; (w,x) interleaved per-dc
    so the first accumulation matmuls gate on the least data. `tables`
    shares another side's rope table tiles (pos_q == pos_k fast path)."""
    w_sb, x_sb = [], []
    for dc in range(8):
        w = inp.tile([128, F], BF16, tag=f"{side}w{dc}")
        nc.sync.dma_start(w[:], w_dram[dc * 128:(dc + 1) * 128, :])
        w_sb.append(w)
        x = inp.tile([128, L], BF16, tag=f"{side}x{dc}")
        nc.sync.dma_start(x[:], x_dram[dc * 128:(dc + 1) * 128, :])
        x_sb.append(x)
    wv_sb = []
    if wv_dram is not None:
        for dc in range(8):
            w = inp.tile([128, F], BF16, tag=f"{side}wv{dc}")
            nc.sync.dma_start(w[:], wv_dram[dc * 128:(dc + 1) * 128, :])
            wv_sb.append(w)
    if tables is not None:
        c_sb, s_sb = tables
    else:
        c_sb = inp.tile([128, L], BF16, tag=f"{side}ctab")
        nc.sync.dma_start(c_sb[:], c_dram[:])
        s_sb = inp.tile([128, L], BF16, tag=f"{side}stab")
        nc.sync.dma_start(s_sb[:], s_dram[:])
    return dict(w=w_sb, x=x_sb, wv=wv_sb, c=c_sb, s=s_sb)


def _proj_compute(tc, tiles, tmp, pps, dst, bdiag, bmap, perm, side,
                  rk_dram=None, vaug=None):
    """Project x (via w) into feature-major bf16 dst tiles [128, L] x4, with
    RMSNorm + RoPE applied. Software-pipelined over 16 chunks [128, 512].

    side == "q": multiply rstd into dst (via broadcast matmul).
    side == "k": write 0.125*rstd chunks to rk_dram instead (consumed by exp),
                 and also project v (tiles["wv"]) into vaug tiles.
    """
    nc = tc.nc
    w_sb, x_sb, wv_sb = tiles["w"], tiles["x"], tiles["wv"]
    c_sb, s_sb = tiles["c"], tiles["s"]
    eps_t = tmp.tile([2, 1], F32, tag=f"eps_{side}")
    nc.gpsimd.memset(eps_t[:], EPS if side == "q" else 64.0 * EPS)

    chunks = [(fb, qc) for fb in range(4) for qc in range(4)]
    st = [dict() for _ in chunks]   # per-chunk pipeline state

    def stage1(c):
        fb, qc = chunks[c]
        col0 = qc * 512
        ps = pps.tile([128, 512], F32, tag="proj", bufs=2)
        for dc in range(8):
            nc.tensor.matmul(ps[:],
                             w_sb[dc][:, fb * 128:(fb + 1) * 128],
                             x_sb[dc][:, col0:col0 + 512],
                             start=(dc == 0), stop=(dc == 7))
        raw = tmp.tile([128, 512], BF16, tag="raw", bufs=2)
        nc.vector.tensor_copy(raw[:], ps[:])          # cast for perm matmul
        sq = tmp.tile([128, 512], BF16, tag="sq", bufs=2)
        nc.scalar.activation(sq[:], ps[:], SQUARE)
        t1 = tmp.tile([128, 512], BF16, tag="t1", bufs=2)
        nc.gpsimd.tensor_mul(t1[:], raw[:], c_sb[:, col0:col0 + 512])
        st[c].update(ps=ps, raw=raw, sq=sq, t1=t1, col0=col0, fb=fb)

    def stage1v(kc):
        # v projection chunk kc -> vaug[kc] (seq-major), k side only
        ps = pps.tile([128, 512], F32, tag="aux", bufs=2)
        for dc in range(8):
            nc.tensor.matmul(ps[:],
                             x_sb[dc][:, kc * 128:(kc + 1) * 128],
                             wv_sb[dc][:],
                             start=(dc == 0), stop=(dc == 7))
        va = vaug[kc]
        nc.gpsimd.memset(va[:], 1.0)
        va3 = va.rearrange("p (h c) -> p h c", c=65)
        ps3 = ps.rearrange("p (h c) -> p h c", c=64)
        nc.vector.tensor_copy(va3[:, :, 0:64], ps3[:])

    def stage2(c):
        s = st[c]
        fb, col0 = s["fb"], s["col0"]
        rot = pps.tile([128, 512], F32, tag="rot", bufs=2)
        nc.tensor.matmul(rot[:], perm[:], s["raw"][:], start=True, stop=True)
        vps = pps.tile([2, 512], F32, tag="var", bufs=2)
        nc.tensor.matmul(vps[:], bdiag[:], s["sq"][:], start=True, stop=True)
        std = tmp.tile([2, 512], F32, tag="std", bufs=2)
        if side == "q":
            # std = sqrt(raw/64 + eps); rstd = 1/std
            nc.scalar.activation(std[:], vps[:], SQRT,
                                 bias=eps_t[:], scale=1.0 / 64.0)
        else:
            # fold the 1/8 attention scale: rk = 1/(8*std) = 1/sqrt(64*(raw/64+eps))
            nc.scalar.activation(std[:], vps[:], SQRT,
                                 bias=eps_t[:], scale=1.0)
        t2 = tmp.tile([128, 512], BF16, tag="t2", bufs=2)
        nc.vector.tensor_mul(t2[:], rot[:], s_sb[:, col0:col0 + 512])
        if side == "q":
            rstd = tmp.tile([2, 512], F32, tag="rstd", bufs=2)
            nc.vector.reciprocal_approx_fast(out=rstd[:], in_=std[:])
            rstd_b = tmp.tile([2, 512], BF16, tag="rstdb", bufs=2)
            nc.scalar.activation(rstd_b[:], rstd[:], COPYF)
            pre = tmp.tile([128, 512], BF16, tag="pre", bufs=2)
            nc.gpsimd.tensor_add(pre[:], s["t1"][:], t2[:])
            s.update(rstd_b=rstd_b, pre=pre)
        else:
            rstd = tmp.tile([2, 512], F32, tag="rstd", bufs=2)
            nc.vector.reciprocal_approx_fast(out=rstd[:], in_=std[:])
            # issue on the gpsimd queue: a data-dependent DMA on the Sync
            # queue would head-of-line-block phase B's input loads
            nc.gpsimd.dma_start(
                rk_dram[2 * fb:2 * fb + 2, col0:col0 + 512], rstd[:])
            nc.vector.tensor_add(dst[fb][:, col0:col0 + 512], s["t1"][:], t2[:])

    def stage3(c):
        # q only: broadcast rstd over the 2x64 head rows and multiply in
        s = st[c]
        fb, col0 = s["fb"], s["col0"]
        bps = pps.tile([128, 512], F32, tag="aux", bufs=2)
        nc.tensor.matmul(bps[:], bmap[:], s["rstd_b"][:], start=True, stop=True)
        nc.vector.tensor_mul(dst[fb][:, col0:col0 + 512], s["pre"][:], bps[:])
        st[c] = {}

    n = len(chunks)
    if side == "k":
        for i in range(n + 1):
            if i < n:
                stage1(i)
                stage1v(i)
            if i >= 1:
                stage2(i - 1)
    else:
        for i in range(n + 2):
            if i < n:
                stage1(i)
            if 1 <= i <= n:
                stage2(i - 1)
            if i >= 2:
                stage3(i - 2)


def _build_program(share_tables):
    nc = bacc.Bacc("TRN2", target_bir_lowering=False, debug=False,
                   num_devices=N_CORES)
    dt = nc.dram_tensor
    xqT = dt("xqT", (D, L), BF16, kind="ExternalInput").ap()
    xkvT = dt("xkvT", (D, L), BF16, kind="ExternalInput").ap()
    wq = dt("wq", (D, F), BF16, kind="ExternalInput").ap()
    wk = dt("wk", (D, F), BF16, kind="ExternalInput").ap()
    wv = dt("wv", (D, F), BF16, kind="ExternalInput").ap()
    wout = dt("wout", (F, D), BF16, kind="ExternalInput").ap()
    cq = dt("cq", (128, L), BF16, kind="ExternalInput").ap()
    sq_t = dt("sq", (128, L), BF16, kind="ExternalInput").ap()
    ck = dt("ck", (128, L), BF16, kind="ExternalInput").ap()
    sk_t = dt("sk", (128, L), BF16, kind="ExternalInput").ap()
    bdiag_d = dt("bdiag", (128, 2), BF16, kind="ExternalInput").ap()
    bmap_d = dt("bmap", (2, 128), BF16, kind="ExternalInput").ap()
    perm_d = dt("perm", (128, 128), BF16, kind="ExternalInput").ap()
    sel_d = [dt(f"sel{i}", (128, 128), BF16, kind="ExternalInput").ap()
             for i in range(2)]
    outT = dt("outT", (D, L), BF16, kind="ExternalOutput").ap()

    with tile.TileContext(nc) as tc:
        with ExitStack() as ctx:
            big = ctx.enter_context(tc.tile_pool(name="big", bufs=1))
            dram = ctx.enter_context(tc.tile_pool(name="dram", bufs=1, space="DRAM"))

            kT = [big.tile([128, L], BF16, tag=f"kT{i}", name=f"kT{i}") for i in range(4)]
            qT = [big.tile([128, L], BF16, tag=f"qT{i}", name=f"qT{i}") for i in range(4)]
            vaug = [big.tile([128, HC * 65], BF16, tag=f"v{i}", name=f"vaug{i}") for i in range(16)]
            rk_dram = dram.tile([HC, L], F32, tag="rk")
            # softmax denominators: per head-group tile, head h at partition
            # row 32*(h%4); 1/sums in bf16 for the phase-D broadcast
            sums_g = [big.tile([128, L], F32, tag=f"sums{g}", name=f"sums{g}")
                      for g in range(2)]
            nc.gpsimd.memset(sums_g[0][:], 1.0)
            nc.gpsimd.memset(sums_g[1][:], 1.0)
            rs_g = [big.tile([128, L], BF16, tag=f"rs{g}", name=f"rs{g}")
                    for g in range(2)]
            rk_sb = big.tile([128, HC, 16], F32, tag="rk_sb")

            bdiag = big.tile([128, 2], BF16, tag="bdiag")
            nc.sync.dma_start(bdiag[:], bdiag_d[:])
            bmap = big.tile([2, 128], BF16, tag="bmap")
            nc.sync.dma_start(bmap[:], bmap_d[:])
            perm = big.tile([128, 128], BF16, tag="perm")
            nc.sync.dma_start(perm[:], perm_d[:])

            # ---- Phases A+B: projections ----
            with ExitStack() as pctx:
                inp = pctx.enter_context(tc.tile_pool(name="inp", bufs=1))
                tmp = pctx.enter_context(tc.tile_pool(name="tmp", bufs=1))
                pps = pctx.enter_context(
                    tc.tile_pool(name="proj_ps", bufs=1, space="PSUM"))
                a_tiles = _proj_dmas(nc, inp, xkvT, wk, ck, sk_t, "k",
                                     wv_dram=wv)
                b_tiles = _proj_dmas(
                    nc, inp, xqT, wq, cq, sq_t, "q",
                    tables=((a_tiles["c"], a_tiles["s"])
                            if share_tables else None))
                _proj_compute(tc, a_tiles, tmp, pps, kT, bdiag, bmap, perm,
                              side="k", rk_dram=rk_dram, vaug=vaug)
                _proj_compute(tc, b_tiles, tmp, pps, qT, bdiag, bmap, perm,
                              side="q")

            # rk transpose gather (Sync queue, after all input loads)
            nc.sync.dma_start(
                rk_sb[:], rk_dram.rearrange("h (kc p) -> p h kc", p=128))

            ytr = [big.tile([128, L], BF16, tag=f"ytr{i}", name=f"ytr{i}")
                   for i in range(4)]

            # ---- Phases C+D ----
            with ExitStack() as cctx:
                cpool = cctx.enter_context(tc.tile_pool(name="cd_sb", bufs=1))
                cps = cctx.enter_context(
                    tc.tile_pool(name="att_ps", bufs=1, space="PSUM"))
                wo_sb = []
                for fc in range(4):
                    w = cpool.tile([128, D], BF16, tag=f"wo{fc}")
                    nc.sync.dma_start(w[:], wout[fc * 128:(fc + 1) * 128, :])
                    wo_sb.append(w)
                sel_sb = []
                for i in range(2):
                    s = cpool.tile([128, 128], BF16, tag=f"sel{i}")
                    nc.sync.dma_start(s[:], sel_d[i][:])
                    sel_sb.append(s)

                # ---- Phase C: attention ----
                va3s = [vaug[kc].rearrange("p (h c) -> p h c", c=65)
                        for kc in range(16)]
                for h in range(HC):
                    fb, off = h // 2, (h % 2) * 64
                    yps = [cps.tile([128, 512], F32, tag=f"y{qn}", bufs=1,
                                    name=f"yps{h}_{qn}")[0:65, :]
                           for qn in range(4)]
                    pend = None   # (kc, [pt_half0, pt_half1]) awaiting attnv
                    for kc in range(16):
                        pts = []
                        for half in range(2):
                            sps = cps.tile([128, 1024], F32, tag="s", bufs=2)
                            for j in range(2):
                                qn = half * 2 + j
                                nc.tensor.matmul(
                                    sps[:, j * 512:(j + 1) * 512],
                                    kT[fb][off:off + 64,
                                           kc * 128:(kc + 1) * 128],
                                    qT[fb][off:off + 64,
                                           qn * 512:(qn + 1) * 512],
                                    start=True, stop=True)
                            pt = cpool.tile([128, 1024], BF16, tag="p", bufs=4)
                            nc.scalar.activation(pt[:], sps[:], EXP,
                                                 scale=rk_sb[:, h, kc:kc + 1])
                            pts.append(pt)
                        if pend is not None:
                            pkc, ppts = pend
                            for half in range(2):
                                for j in range(2):
                                    qn = half * 2 + j
                                    nc.tensor.matmul(
                                        yps[qn][:], va3s[pkc][:, h, :],
                                        ppts[half][:, j * 512:(j + 1) * 512],
                                        start=(pkc == 0), stop=False)
                        pend = (kc, pts)
                    pkc, ppts = pend
                    for half in range(2):
                        for j in range(2):
                            qn = half * 2 + j
                            nc.tensor.matmul(
                                yps[qn][:], va3s[pkc][:, h, :],
                                ppts[half][:, j * 512:(j + 1) * 512],
                                start=False, stop=True)
                    slot = 32 * (h % 4)
                    g = h // 4
                    for qn in range(4):
                        sl = slice(qn * 512, (qn + 1) * 512)
                        nc.vector.tensor_copy(ytr[fb][off:off + 64, sl],
                                              yps[qn][0:64, :])
                        nc.vector.tensor_copy(sums_g[g][slot:slot + 1, sl],
                                              yps[qn][64:65, :])
                        if h in (3, 7):
                            # group complete: fold 1/sums per qn chunk while
                            # attention (or phase D stage 1) is still pending
                            rs32 = cpool.tile([128, 512], F32, tag="rs32",
                                              bufs=2)
                            nc.vector.reciprocal_approx_fast(
                                out=rs32[:], in_=sums_g[g][:, sl])
                            nc.vector.tensor_copy(rs_g[g][:, sl], rs32[:])

                # ---- Phase D: normalize + output projection (per-qn) ----
                def d_stage1(qn):
                    sl = slice(qn * 512, (qn + 1) * 512)
                    for pair in range(2):
                        bt = cps.tile([128, 1024], F32, tag="s", bufs=2,
                                      name=f"bc2_{pair}_{qn}")
                        for half in range(2):
                            fb = pair * 2 + half
                            bps = bt[:, half * 512:(half + 1) * 512]
                            nc.tensor.matmul(bps, sel_sb[fb % 2][:],
                                             rs_g[fb // 2][:, sl],
                                             start=True, stop=True)
                            nc.vector.tensor_mul(ytr[fb][:, sl],
                                                 ytr[fb][:, sl], bps)

                def d_stage2(qn):
                    sl = slice(qn * 512, (qn + 1) * 512)
                    for nb in range(8):
                        ps = cps.tile([128, 512], F32, tag=f"y{nb % 4}",
                                      bufs=1, name=f"oproj_{nb}_{qn}")
                        for fc in range(4):
                            nc.tensor.matmul(
                                ps[:],
                                wo_sb[fc][:, nb * 128:(nb + 1) * 128],
                                ytr[fc][:, sl],
                                start=(fc == 0), stop=(fc == 3))
                        ot = cpool.tile([128, 512], BF16, tag="ot", bufs=4)
                        nc.vector.tensor_copy(ot[:], ps[:])
                        eng = nc.sync if nb % 2 == 0 else nc.gpsimd
                        eng.dma_start(
                            outT[nb * 128:(nb + 1) * 128, sl], ot[:])

                for i in range(5):
                    if i < 4:
                        d_stage1(i)
                    if i >= 1:
                        d_stage2(i - 1)
    nc.compile()
    return nc


def get_nc(share_tables=True):
    global _NC
    if _NC is None or _NC[1] != share_tables:
        _NC = (_build_program(share_tables), share_tables)
    return _NC[0]


# --------------------------------------------------------------------------- #
# Host side
# --------------------------------------------------------------------------- #

def _rope_tables(pos, g):
    """Feature-major folded RoPE(+gain) tables, replicated for a 2-head tile."""
    pos = np.asarray(pos).astype(np.float32)
    g = np.asarray(g, dtype=np.float32)
    inv = (1.0 / (10000.0 ** (np.arange(0, DH, 2, dtype=np.float32)
                              / np.float32(DH)))).astype(np.float32)
    ang = pos[:, None] * inv[None, :]                      # (L, 32)
    cos, sin = np.cos(ang, dtype=np.float32), np.sin(ang, dtype=np.float32)
    j = np.arange(DH)
    C = (g[j][:, None] * cos[:, j % 32].T).astype(np.float32)       # (64, L)
    sign = np.where(j < 32, -1.0, 1.0).astype(np.float32)
    S = (sign[:, None] * g[(j + 32) % 64][:, None]
         * sin[:, j % 32].T).astype(np.float32)
    return (np.ascontiguousarray(np.tile(C, (2, 1))).astype(NP_BF16),
            np.ascontiguousarray(np.tile(S, (2, 1))).astype(NP_BF16))


def make_in_maps(queries, kv, Wq, Wkv, Wout, g_q, g_k, pos_q, pos_k):
    queries = np.asarray(queries, dtype=np.float32)
    kv = np.asarray(kv, dtype=np.float32)
    Wq = np.asarray(Wq, dtype=np.float32)
    Wkv = np.asarray(Wkv, dtype=np.float32)
    Wout = np.asarray(Wout, dtype=np.float32)

    cq, sq = _rope_tables(pos_q, g_q)
    ck, sk = _rope_tables(pos_k, g_k)
    bdiag = np.zeros((128, 2), np.float32)
    bdiag[0:64, 0] = 1.0
    bdiag[64:128, 1] = 1.0
    bmap = np.zeros((2, 128), np.float32)
    bmap[0, 0:64] = 1.0
    bmap[1, 64:128] = 1.0
    # unsigned rotate-half permutation (sign lives in the S table):
    # rot[i] = raw[i+32] for i%64<32 else raw[i-32]
    perm = np.zeros((128, 128), np.float32)
    for i in range(128):
        src = i + 32 if (i % 64) < 32 else i - 32
        perm[src, i] = 1.0
    # sums-row selectors: within its group tile, head h's denominators live
    # at row 32*(h%4); ytr[fb] rows 0:64 = head 2fb, 64:128 = head 2fb+1
    selA = np.zeros((128, 128), np.float32)
    selA[0, 0:64] = 1.0
    selA[32, 64:128] = 1.0
    selB = np.zeros((128, 128), np.float32)
    selB[64, 0:64] = 1.0
    selB[96, 64:128] = 1.0

    Wkv3 = Wkv.reshape(D, 16, 2 * DH)
    in_maps = []
    for c in range(N_CORES):
        b, grp = c // 2, c % 2
        hs = slice(grp * HC, (grp + 1) * HC)
        im = {
            "xqT": np.ascontiguousarray(queries[b].T).astype(NP_BF16),
            "xkvT": np.ascontiguousarray(kv[b].T).astype(NP_BF16),
            "wq": np.ascontiguousarray(
                Wq[:, grp * F:(grp + 1) * F]).astype(NP_BF16),
            "wk": np.ascontiguousarray(
                Wkv3[:, hs, :DH].reshape(D, F)).astype(NP_BF16),
            "wv": np.ascontiguousarray(
                Wkv3[:, hs, DH:].reshape(D, F)).astype(NP_BF16),
            "wout": np.ascontiguousarray(
                Wout[grp * F:(grp + 1) * F, :]).astype(NP_BF16),
            "cq": cq, "sq": sq, "ck": ck, "sk": sk,
            "bdiag": bdiag.astype(NP_BF16), "bmap": bmap.astype(NP_BF16),
            "perm": perm.astype(NP_BF16),
            "sel0": selA.astype(NP_BF16), "sel1": selB.astype(NP_BF16),
        }
        in_maps.append(im)
    return in_maps


def kernel(queries, kv, Wq, Wkv, Wout, g_q, g_k, pos_q, pos_k):
    global LAST_RESULTS
    share = bool(np.array_equal(np.asarray(pos_q), np.asarray(pos_k))
                 and np.array_equal(np.asarray(g_q), np.asarray(g_k)))
    nc = get_nc(share)
    in_maps = make_in_maps(queries, kv, Wq, Wkv, Wout, g_q, g_k, pos_q, pos_k)
    trace = bool(int(os.environ.get("KERNEL_TRACE", "0")))
    kw = {}
    if trace:
        kw["tmpdir"] = os.environ.get("KERNEL_TRACE_DIR") or None
    res = run_bass_kernel_spmd(nc, in_maps, core_ids=list(range(N_CORES)),
                               trace=trace, **kw)
    LAST_RESULTS = res
    out = np.empty((4, L, D), np.float32)
    for b in range(4):
        out[b] = (res.results[2 * b]["outT"].astype(np.float32)
                  + res.results[2 * b + 1]["outT"].astype(np.float32)).T
    return out

